# revision 15
# baseline (speedup 1.0000x reference)
"""Trainium2 Bass kernel for nn_MinimalBeatDecoder (nms_detection).

Reference semantics: peaks = positive local maxima of a 7-wide window over a
16.7M-frame logit stream; runs of index-adjacent peaks merge into sections;
output = averaged frame index of the first 2^21 sections, padded with -1.

Strategy (sequence-parallel over 8 NeuronCores, 2^21 frames each):
  - per core, frames laid out as 128 rows x 16384, processed in chunks with
    an 8-frame halo handled via overlapping DMA rows.
  - the DVE computes pair maxes m2[s] = max(x[2s], x[2s+1]) (strided fp32
    reads, bf16 output; fp32->bf16 rounding is monotone so order relations
    survive up to ties), then ch[s] = max(m2[s-1], m2[s+1]), then a single
    uint8 candidate mask pk[s] = m2[s] >= ch[s].
  - every true peak p is the max of its own pair and >= every element of the
    two neighbouring pairs (all lie within its 7-wide window), so pk flags
    that pair: the mask is a guaranteed superset (~1/3 of pairs, local maxima
    of the pair-max sequence). The mask streams back to HBM (1MB/core).
  - the host expands candidate pairs to positions and verifies each against
    the exact fp32 rule (x > 0 and x >= its 6 neighbours), then applies the
    exact merge/average section semantics on the sparse peak list. The kernel
    is therefore exact for arbitrary inputs; the device mask is only a
    conservative prefilter.
"""

import sys

sys.path.insert(0, "/opt/trn_rl_repo")

import numpy as np

import concourse.bacc as bacc
import concourse.bass as bass
import concourse.mybir as mybir
import concourse.tile as tile
from concourse import bass_utils

# geometry
NCORES = 8
NFRAMES = 16_777_216
PERCORE = NFRAMES // NCORES  # 2^21
MAX_BEATS = NFRAMES // 8  # 2^21
MERGE_INTERVAL = 1

P = 128  # partitions
W = PERCORE // P  # 16384 frames per row
HALO = 8  # left 4 + right 4 extra frames per row load
# compute chunks (frame offset in row, width); small first/last to ramp/drain
CHUNKS = [(0, 512), (512, 1024), (1536, 2048), (3584, 2048), (5632, 2048),
          (7680, 2048), (9728, 2048), (11776, 2048), (13824, 2048),
          (15872, 512)]
# input DMA slices (frame offset, width); the queue is in-order so slice k's
# completion fires at its cumulative-bytes point in the stream
SLICES = [(0, 256), (256, 768), (1024, 2048), (3072, 2560), (5632, 3072),
          (8704, 3584), (12288, 3328), (15616, 768)]
# mask store splits (pair-col offset, width); small final store keeps the
# drain off the critical path
MSTORES = [(0, 4096), (4096, 2048), (6144, 1792), (7936, 256)]

F32 = mybir.dt.float32
BF16 = mybir.dt.bfloat16
U8 = mybir.dt.uint8

NEG_BIG = -3.0e38  # halo fill; below any logit, representable in bf16


def build_kernel(p=P, w=W):
    """Per-core SPMD program. Inputs:
      xin [p*w + HALO] f32  (frame t of this core at index t+4)
    Outputs:
      mp [p, w//2] u8  (pair-level candidate mask)
    """
    nc = bacc.Bacc("TRN2", target_bir_lowering=False)
    xin = nc.dram_tensor("xin", [p * w + HALO], F32, kind="ExternalInput")
    # pair-candidate mask, u8 payload shipped as f32 words (u8-element DMA
    # descriptors run ~3x slower than 4B-element ones)
    mp = nc.dram_tensor("mp", [p, w // 8], F32, kind="ExternalOutput")

    with tile.TileContext(nc) as tc:
        with (
            tc.tile_pool(name="io", bufs=1) as io_pool,
            tc.tile_pool(name="bfw", bufs=3) as bf_pool,
            tc.tile_pool(name="msk", bufs=3) as mk_pool,
        ):
            # whole-row resident input tile; slice DMAs land independently so
            # compute trails the stream without buffer-recycle stalls.
            # tile col t holds frame t-4 of this core-row (halo included).
            xr = io_pool.tile([p, w + HALO], F32, tag="xr")
            for off, sw in SLICES:
                if off + sw == w:
                    sw += HALO
                src = bass.AP(
                    tensor=xin,
                    offset=off,
                    ap=[[w, p], [1, sw]],
                )
                nc.sync.dma_start(xr[:, off : off + sw], src)

            # resident pair-candidate mask; is_ge writes straight into it
            pkr = mk_pool.tile([p, w // 2], U8, tag="pkr")
            mdone = 0
            for off, cw in CHUNKS:
                hw = cw // 2
                # pair maxes with +-1 pair halo: m2h[u] = m2 of pair
                # (off/2 + u - 1); reads tile cols [off+2, off+cw+6)
                m2h = bf_pool.tile([p, hw + 2], BF16, tag="m2h")
                nc.vector.tensor_tensor(
                    out=m2h[:], in0=xr[:, off + 2 : off + cw + 6 : 2],
                    in1=xr[:, off + 3 : off + cw + 6 : 2],
                    op=mybir.AluOpType.max,
                )
                # ch[v] = max(m2[v-1], m2[v+1])
                ch = bf_pool.tile([p, hw], BF16, tag="ch")
                nc.vector.tensor_tensor(
                    out=ch[:], in0=m2h[:, 0:hw], in1=m2h[:, 2 : hw + 2],
                    op=mybir.AluOpType.max,
                )
                # pk[v] = m2[v] >= max(m2[v-1], m2[v+1])
                ho = off // 2
                nc.vector.tensor_tensor(
                    out=pkr[:, ho : ho + hw], in0=m2h[:, 1 : hw + 1],
                    in1=ch[:], op=mybir.AluOpType.is_ge,
                )
                # flush finished mask spans
                while mdone < len(MSTORES) and MSTORES[mdone][0] + MSTORES[mdone][1] <= ho + hw:
                    mo, mw = MSTORES[mdone]
                    nc.scalar.dma_start(
                        mp[:, mo // 4 : (mo + mw) // 4],
                        pkr[:, mo : mo + mw].bitcast(F32),
                    )
                    mdone += 1
    nc.compile()
    return nc


_cached = {}


def _get_nc():
    if "nc" not in _cached:
        _cached["nc"] = build_kernel()
    return _cached["nc"]


def _host_reference_fallback(x):
    """Exact numpy reference (kept for test harness use)."""
    n = x.shape[0]
    import numpy.lib.stride_tricks as st

    xp = np.pad(x, (3, 3), constant_values=-np.inf)
    pooled = st.sliding_window_view(xp, 7).max(axis=1)
    peak = (x == pooled) & (x > 0)
    idx = np.arange(n, dtype=np.int64)
    prev = np.concatenate([[False], peak[:-1]])
    is_new = peak & ~prev
    sec = np.cumsum(is_new) - 1
    sums = np.zeros(MAX_BEATS + 1, np.float64)
    cnts = np.zeros(MAX_BEATS + 1, np.float64)
    sel = peak & (sec < MAX_BEATS)
    np.add.at(sums, sec[sel], idx[sel].astype(np.float64))
    np.add.at(cnts, sec[sel], 1.0)
    out = np.full(MAX_BEATS, -1.0, np.float32)
    m = cnts[:MAX_BEATS] > 0
    out[m] = (sums[:MAX_BEATS][m] / cnts[:MAX_BEATS][m]).astype(np.float32)
    return out[None, :]


def kernel(logit: np.ndarray) -> np.ndarray:
    x = np.asarray(logit, dtype=np.float32)[0]

    nc = _get_nc()

    xpad = np.full(NFRAMES + 8, np.float32(NEG_BIG), dtype=np.float32)
    xpad[4 : 4 + NFRAMES] = x

    in_maps = []
    for c in range(NCORES):
        base = c * PERCORE
        in_maps.append(
            {"xin": np.ascontiguousarray(xpad[base : base + PERCORE + HALO])}
        )

    global _last_in_maps
    _last_in_maps = in_maps
    res = bass_utils.run_bass_kernel_spmd(
        nc, in_maps, core_ids=list(range(NCORES))
    )

    # host: candidate pairs -> positions (globally sorted)
    pair_parts = []
    for c in range(NCORES):
        m = np.ascontiguousarray(res.results[c]["mp"]).view(np.uint8)
        k = np.flatnonzero(m)  # flat idx == pair idx in core
        pair_parts.append(k.astype(np.int64) + c * (PERCORE // 2))
    pairs = np.concatenate(pair_parts)

    # each candidate pair contributes both its positions; verify exactly
    cand = np.empty(2 * pairs.size, dtype=np.int64)
    cand[0::2] = 2 * pairs
    cand[1::2] = 2 * pairs + 1
    cx = xpad[cand + 4]
    ok = cx > 0
    for d in (1, 2, 3):
        ok &= cx >= xpad[cand + 4 - d]
        ok &= cx >= xpad[cand + 4 + d]
    peaks = cand[ok]

    # exact section semantics on the sparse peak list: peaks with gap
    # <= MERGE_INTERVAL merge into one section, averaged position
    out = np.full(MAX_BEATS, -1.0, dtype=np.float32)
    if peaks.size:
        gap = np.diff(peaks)
        starts = np.flatnonzero(np.concatenate(([True], gap > MERGE_INTERVAL)))
        sums = np.add.reduceat(peaks.astype(np.float64), starts)
        cnts = np.diff(np.concatenate((starts, [peaks.size])))
        beats = (sums / cnts).astype(np.float32)[:MAX_BEATS]
        out[: beats.size] = beats
    return out[None, :]


# revision 17
# speedup vs baseline: 1.0699x; 1.0699x over previous
"""Trainium2 Bass kernel for nn_MinimalBeatDecoder (nms_detection).

Reference semantics: peaks = positive local maxima of a 7-wide window over a
16.7M-frame logit stream; runs of index-adjacent peaks merge into sections;
output = averaged frame index of the first 2^21 sections, padded with -1.

Strategy (sequence-parallel over 8 NeuronCores, 2^21 frames each):
  - per core, frames laid out as 128 rows x 16384, processed in chunks with
    an 8-frame halo handled via overlapping DMA rows.
  - the DVE computes pair maxes m2[s] = max(x[2s], x[2s+1]) (strided fp32
    reads, bf16 output; fp32->bf16 rounding is monotone so order relations
    survive up to ties), then ch[s] = max(m2[s-1], m2[s+1]), then a single
    uint8 candidate mask pk[s] = m2[s] >= ch[s].
  - every true peak p is the max of its own pair and >= every element of the
    two neighbouring pairs (all lie within its 7-wide window), so pk flags
    that pair: the mask is a guaranteed superset (~1/3 of pairs, local maxima
    of the pair-max sequence). The mask streams back to HBM (1MB/core).
  - the host expands candidate pairs to positions and verifies each against
    the exact fp32 rule (x > 0 and x >= its 6 neighbours), then applies the
    exact merge/average section semantics on the sparse peak list. The kernel
    is therefore exact for arbitrary inputs; the device mask is only a
    conservative prefilter.
"""

import sys

sys.path.insert(0, "/opt/trn_rl_repo")

import numpy as np

import concourse.bacc as bacc
import concourse.bass as bass
import concourse.mybir as mybir
import concourse.tile as tile
from concourse import bass_utils

# geometry
NCORES = 8
NFRAMES = 16_777_216
PERCORE = NFRAMES // NCORES  # 2^21
MAX_BEATS = NFRAMES // 8  # 2^21
MERGE_INTERVAL = 1

P = 128  # partitions
W = PERCORE // P  # 16384 frames per row
HALO = 8  # left 4 + right 4 extra frames per row load
# compute chunks (frame offset in row, width); small first/last to ramp/drain
CHUNKS = [(0, 512), (512, 1024), (1536, 2048), (3584, 2048), (5632, 2048),
          (7680, 2048), (9728, 2048), (11776, 2048), (13824, 2048),
          (15872, 512)]
# input DMA slices (tile-col offset, width), cut at chunk-boundary + HALO so
# chunk k's read range [off_k, off_k+cw+8) is covered by slices 0..k exactly
# (the queue is in-order, so slice k completes at its cumulative-bytes point)
SLICES = [(0, 520), (520, 1024), (1544, 2048), (3592, 2048), (5640, 2048),
          (7688, 2048), (9736, 2048), (11784, 2048), (13832, 2048),
          (15880, 512)]
# mask store splits (pair-col offset, width); small final store keeps the
# drain off the critical path
MSTORES = [(0, 4096), (4096, 2048), (6144, 1792), (7936, 256)]

F32 = mybir.dt.float32
BF16 = mybir.dt.bfloat16
U8 = mybir.dt.uint8

NEG_BIG = -3.0e38  # halo fill; below any logit, representable in bf16


def build_kernel(p=P, w=W):
    """Per-core SPMD program. Inputs:
      xin [p*w + HALO] f32  (frame t of this core at index t+4)
    Outputs:
      mp [p, w//2] u8  (pair-level candidate mask)
    """
    nc = bacc.Bacc("TRN2", target_bir_lowering=False)
    xin = nc.dram_tensor("xin", [p * w + HALO], F32, kind="ExternalInput")
    # pair-candidate mask, u8 payload shipped as f32 words (u8-element DMA
    # descriptors run ~3x slower than 4B-element ones)
    mp = nc.dram_tensor("mp", [p, w // 8], F32, kind="ExternalOutput")

    with tile.TileContext(nc) as tc:
        with (
            tc.tile_pool(name="io", bufs=1) as io_pool,
            tc.tile_pool(name="bfw", bufs=3) as bf_pool,
            tc.tile_pool(name="msk", bufs=3) as mk_pool,
        ):
            # whole-row resident input tile; slice DMAs land independently so
            # compute trails the stream without buffer-recycle stalls.
            # tile col t holds frame t-4 of this core-row (halo included).
            xr = io_pool.tile([p, w + HALO], F32, tag="xr")
            for off, sw in SLICES:
                src = bass.AP(
                    tensor=xin,
                    offset=off,
                    ap=[[w, p], [1, sw]],
                )
                nc.sync.dma_start(xr[:, off : off + sw], src)

            # resident pair-candidate mask; is_ge writes straight into it
            pkr = mk_pool.tile([p, w // 2], U8, tag="pkr")
            mdone = 0
            for off, cw in CHUNKS:
                hw = cw // 2
                # pair maxes with +-1 pair halo: m2h[u] = m2 of pair
                # (off/2 + u - 1); reads tile cols [off+2, off+cw+6)
                m2h = bf_pool.tile([p, hw + 2], BF16, tag="m2h")
                nc.vector.tensor_tensor(
                    out=m2h[:], in0=xr[:, off + 2 : off + cw + 6 : 2],
                    in1=xr[:, off + 3 : off + cw + 6 : 2],
                    op=mybir.AluOpType.max,
                )
                # ch[v] = max(m2[v-1], m2[v+1])
                ch = bf_pool.tile([p, hw], BF16, tag="ch")
                nc.vector.tensor_tensor(
                    out=ch[:], in0=m2h[:, 0:hw], in1=m2h[:, 2 : hw + 2],
                    op=mybir.AluOpType.max,
                )
                # pk[v] = m2[v] >= max(m2[v-1], m2[v+1])
                ho = off // 2
                nc.vector.tensor_tensor(
                    out=pkr[:, ho : ho + hw], in0=m2h[:, 1 : hw + 1],
                    in1=ch[:], op=mybir.AluOpType.is_ge,
                )
                # flush finished mask spans
                while mdone < len(MSTORES) and MSTORES[mdone][0] + MSTORES[mdone][1] <= ho + hw:
                    mo, mw = MSTORES[mdone]
                    nc.scalar.dma_start(
                        mp[:, mo // 4 : (mo + mw) // 4],
                        pkr[:, mo : mo + mw].bitcast(F32),
                    )
                    mdone += 1
    nc.compile()
    return nc


_cached = {}


def _get_nc():
    if "nc" not in _cached:
        _cached["nc"] = build_kernel()
    return _cached["nc"]


def _host_reference_fallback(x):
    """Exact numpy reference (kept for test harness use)."""
    n = x.shape[0]
    import numpy.lib.stride_tricks as st

    xp = np.pad(x, (3, 3), constant_values=-np.inf)
    pooled = st.sliding_window_view(xp, 7).max(axis=1)
    peak = (x == pooled) & (x > 0)
    idx = np.arange(n, dtype=np.int64)
    prev = np.concatenate([[False], peak[:-1]])
    is_new = peak & ~prev
    sec = np.cumsum(is_new) - 1
    sums = np.zeros(MAX_BEATS + 1, np.float64)
    cnts = np.zeros(MAX_BEATS + 1, np.float64)
    sel = peak & (sec < MAX_BEATS)
    np.add.at(sums, sec[sel], idx[sel].astype(np.float64))
    np.add.at(cnts, sec[sel], 1.0)
    out = np.full(MAX_BEATS, -1.0, np.float32)
    m = cnts[:MAX_BEATS] > 0
    out[m] = (sums[:MAX_BEATS][m] / cnts[:MAX_BEATS][m]).astype(np.float32)
    return out[None, :]


def kernel(logit: np.ndarray) -> np.ndarray:
    x = np.asarray(logit, dtype=np.float32)[0]

    nc = _get_nc()

    xpad = np.full(NFRAMES + 8, np.float32(NEG_BIG), dtype=np.float32)
    xpad[4 : 4 + NFRAMES] = x

    in_maps = []
    for c in range(NCORES):
        base = c * PERCORE
        in_maps.append(
            {"xin": np.ascontiguousarray(xpad[base : base + PERCORE + HALO])}
        )

    global _last_in_maps
    _last_in_maps = in_maps
    res = bass_utils.run_bass_kernel_spmd(
        nc, in_maps, core_ids=list(range(NCORES))
    )

    # host: candidate pairs -> positions (globally sorted)
    pair_parts = []
    for c in range(NCORES):
        m = np.ascontiguousarray(res.results[c]["mp"]).view(np.uint8)
        k = np.flatnonzero(m)  # flat idx == pair idx in core
        pair_parts.append(k.astype(np.int64) + c * (PERCORE // 2))
    pairs = np.concatenate(pair_parts)

    # each candidate pair contributes both its positions; verify exactly
    cand = np.empty(2 * pairs.size, dtype=np.int64)
    cand[0::2] = 2 * pairs
    cand[1::2] = 2 * pairs + 1
    cx = xpad[cand + 4]
    ok = cx > 0
    for d in (1, 2, 3):
        ok &= cx >= xpad[cand + 4 - d]
        ok &= cx >= xpad[cand + 4 + d]
    peaks = cand[ok]

    # exact section semantics on the sparse peak list: peaks with gap
    # <= MERGE_INTERVAL merge into one section, averaged position
    out = np.full(MAX_BEATS, -1.0, dtype=np.float32)
    if peaks.size:
        gap = np.diff(peaks)
        starts = np.flatnonzero(np.concatenate(([True], gap > MERGE_INTERVAL)))
        sums = np.add.reduceat(peaks.astype(np.float64), starts)
        cnts = np.diff(np.concatenate((starts, [peaks.size])))
        beats = (sums / cnts).astype(np.float32)[:MAX_BEATS]
        out[: beats.size] = beats
    return out[None, :]
